# revision 12
# baseline (speedup 1.0000x reference)
"""CircleLoss forward on 8 Trainium2 NeuronCores (Bass/Tile).

Math
----
reference computes, with MARGIN=0.4, GAMMA=80:
    prob = clusters @ clusters.T            (binary when clusters is one-hot)
    pos  = strict-upper & (prob > 0)        (same-cluster pairs, j > i)
    neg  = strict-upper & (prob <= 0)
    logit_p = -relu(1.4 - sim) * (sim - 0.6) * 80
    loss = wp_mean * softplus(lse(logit_p over pos))
         + wn_mean * softplus(lse(logit_n over neg))

With one-hot clusters, prob is exactly {0,1}:
    wn_mean = sum(prob over prob<=0)/cnt = 0       -> neg branch vanishes
    wp_mean = cnt_p/cnt_p = 1 (or 0 if no pos pair)
and |sim| < 1.4 (sim = tanh(...)) makes the relu inactive:
    logit_p = 80*(sim-1)^2 - 12.8
So: loss = softplus( log sum_{pos} exp(80*(sim-1)^2 - 12.8) ).

Since (sim-1)^2 <= 4 for sim in [-1, 1], exp(80*sq - 320) <= 1 never
overflows; we use the fixed offset 320 instead of a data max and the
host adds it back:  lse = ln(S) + (320 - 12.8).

Device kernel (SPMD, identical program on 8 cores)
--------------------------------------------------
Core c owns rows [512c, 512c+512), processed as 4 tiles of 128 rows.
Per [128, 4096] tile:
  DVE : em  = (cid_cols == cid_row)                 bf16, 4x mode
  GPS : em2 = affine_select(em, keep j' > p+128t)   strict-upper mask
  ACT : sq  = Square(sim - 1)                       f32
  ACT : e   = Exp(80*sq - 320)                      f32 -> bf16
  DVE : se  = reduce_add(e * em2)                   -> [128, 1] f32
Host sums the 8*[128,4] partials (f64) and applies softplus.

The affine_select base must be a compile-time constant, but the strict
upper triangle depends on the core's global row offset 512c. Fix: each
core's shard is column-ROTATED by -512c (host-side np.roll), so rotated
column j' maps to original j = (j'+512c) % 4096 and the mask condition
becomes j' > 128t + p -- identical on every core. Rotated-in columns
with original j < 512c are always below the diagonal for this core's
rows; the host overwrites their cluster-id with a sentinel (64) so the
equality mask kills them.
"""

import numpy as np

N = 4096
C = 64
NCORES = 8
RPC = N // NCORES          # rows per core = 512
P = 128                    # partitions per tile
MARGIN = 0.4
GAMMA = 80.0
EXP_OFFSET = 320.0         # exp(GAMMA*sq - EXP_OFFSET); sq <= 4 -> arg <= 0
LSE_BACK = EXP_OFFSET - GAMMA * (1.0 - MARGIN) ** 2 * 0.0 - 12.8
# logit = 80*sq - 12.8 ; e = exp(80*sq - 320) = exp(logit - 307.2)
LSE_BACK = EXP_OFFSET - 12.8

_CACHE = {}


def _build_module(n, ncores, rpc):
    """Build the SPMD Bass module (identical program for every core)."""
    import concourse.bacc as bacc
    import concourse.bass as bass
    import concourse.mybir as mybir
    import concourse.tile as tile
    from contextlib import ExitStack

    p = P
    tiles = rpc // p
    assert rpc % p == 0

    nc = bacc.Bacc(
        "TRN2",
        target_bir_lowering=False,
        debug=False,
        num_devices=ncores,
    )
    f32 = mybir.dt.float32
    bf16 = mybir.dt.bfloat16

    # activation() lowers float biases through the const-AP database; only
    # 0.0/1.0 are pre-registered, so register the two biases we use.
    for val in (-1.0, -EXP_OFFSET):
        t = nc.alloc_sbuf_tensor(f"const-f32-{val}", [P, 1], f32)
        nc.gpsimd.memset(t.ap(), val)
        nc.const_aps.aps[(f32, val)] = t.ap()
    nc.all_engine_barrier()

    sim_in = nc.dram_tensor("simrot", [rpc, n], f32, kind="ExternalInput").ap()
    cid_in = nc.dram_tensor("cidrot", [1, n], bf16, kind="ExternalInput").ap()
    cidrow_in = nc.dram_tensor("cidrow", [p, tiles], f32, kind="ExternalInput").ap()
    out = nc.dram_tensor("se_out", [p, tiles], f32, kind="ExternalOutput").ap()

    MASKV = -1.25e7  # additive mask; *GAMMA in exp scale -> exp(-1e9) = 0

    with tile.TileContext(nc) as tc, ExitStack() as ctx:
        consts = ctx.enter_context(tc.tile_pool(name="consts", bufs=1))
        sim_pool = ctx.enter_context(tc.tile_pool(name="sim", bufs=3))
        em_pool = ctx.enter_context(tc.tile_pool(name="em", bufs=2))
        sq_pool = ctx.enter_context(tc.tile_pool(name="sq", bufs=2))
        arg_pool = ctx.enter_context(tc.tile_pool(name="arg", bufs=2))
        e_pool = ctx.enter_context(tc.tile_pool(name="e", bufs=2))

        # tile 0's sim load must be first in the DMA queue — everything
        # else (cid broadcast, row cids) is only needed later
        sim0 = sim_pool.tile([p, n], f32, name="sim0", tag="sim")
        nc.sync.dma_start(out=sim0[:], in_=sim_in[0:p, :])

        # one-time per core: broadcast the rotated cid vector to 128
        # partitions (DMA replication from DRAM), load per-row cids
        cid128 = consts.tile([p, n], bf16)
        nc.sync.dma_start(out=cid128[:], in_=cid_in.partition_broadcast(p))
        cidrow = [
            consts.tile([p, 1], f32, name=f"cr{t}", tag=f"cr{t}")
            for t in range(tiles)
        ]
        for t in range(tiles):
            nc.sync.dma_start(out=cidrow[t][:], in_=cidrow_in[:, t : t + 1])
        se = consts.tile([p, tiles], f32)

        for t in range(tiles):
            if t == 0:
                sim_t = sim0
            else:
                sim_t = sim_pool.tile([p, n], f32, name=f"sim{t}", tag="sim")
                nc.sync.dma_start(
                    out=sim_t[:], in_=sim_in[t * p : (t + 1) * p, :]
                )

            # additive mask: 0 where same cluster, -1.25e7 otherwise
            em = em_pool.tile([p, n], bf16)
            nc.vector.tensor_scalar(
                em[:], cid128[:], cidrow[t][:], MASKV,
                mybir.AluOpType.not_equal, mybir.AluOpType.mult,
            )
            # strict-upper triangle: after rotation only the first
            # 128*(t+1) columns can violate j' > p + 128t -- patch in place
            w = p * (t + 1)
            nc.gpsimd.affine_select(
                out=em[:, 0:w], in_=em[:, 0:w],
                pattern=[[1, w]],
                compare_op=mybir.AluOpType.is_gt,
                fill=MASKV,
                base=-(t * p),
                channel_multiplier=-1,
            )

            sq = sq_pool.tile([p, n], f32)
            nc.scalar.activation(
                sq[:], sim_t[:], mybir.ActivationFunctionType.Square,
                bias=-1.0, scale=1.0,
            )

            argm = arg_pool.tile([p, n], f32)
            nc.vector.tensor_tensor(
                argm[:], sq[:], em[:], mybir.AluOpType.add
            )

            # exp with fused row-accumulate; individual row sums are never
            # needed (fixed offset), so accum over the free dim is the
            # entire per-partition contribution of this tile
            e = e_pool.tile([p, n], bf16)
            nc.scalar.activation(
                e[:], argm[:], mybir.ActivationFunctionType.Exp,
                bias=-EXP_OFFSET, scale=GAMMA,
                accum_out=se[:, t : t + 1],
            )

        nc.sync.dma_start(out=out, in_=se[:])

    nc.compile()
    return nc


def _get_module(n=N, ncores=NCORES, rpc=RPC):
    key = (n, ncores, rpc)
    if key not in _CACHE:
        _CACHE[key] = _build_module(n, ncores, rpc)
    return _CACHE[key]


def make_in_maps(sim, cid, n=N, ncores=NCORES, rpc=RPC):
    """Per-core rotated shards + cid vectors (see module docstring)."""
    import ml_dtypes

    tiles = rpc // P
    in_maps = []
    for c in range(ncores):
        off = c * rpc
        shard = np.roll(sim[off : off + rpc, :], -off, axis=1)
        cidrot = np.roll(cid, -off)
        if off:
            cidrot[n - off :] = C  # sentinel: wrapped cols are below-diagonal
        cidrow = cid[off : off + rpc].reshape(tiles, P).T  # [P, tiles]
        in_maps.append(
            {
                "simrot": np.ascontiguousarray(shard, dtype=np.float32),
                "cidrot": cidrot.reshape(1, n).astype(ml_dtypes.bfloat16),
                "cidrow": np.ascontiguousarray(cidrow).astype(np.float32),
            }
        )
    return in_maps


def _finish(se_arrays, cid):
    """Merge per-core partial sums into the loss (host, f64)."""
    counts = np.bincount(cid, minlength=C)
    cnt_p = int((counts * (counts - 1) // 2).sum())
    if cnt_p == 0:
        return np.float32(0.0)
    S = float(sum(np.asarray(a, dtype=np.float64).sum() for a in se_arrays))
    if not (S > 1e-35):
        return None  # degenerate: all pos terms underflowed; caller falls back
    lse = np.log(S) + LSE_BACK
    loss = np.logaddexp(0.0, lse)  # softplus
    return np.float32(loss)


def _reference_host(sim, clu):
    """Exact fallback (general inputs), numpy float32 to match reference."""
    sim = sim.astype(np.float32)
    prob = (clu @ clu.T).astype(np.float32)
    upper = np.triu(np.ones(sim.shape, dtype=bool), k=1)
    pos = upper & (prob > 0)
    neg = upper & (prob <= 0)
    ap = np.maximum(-sim + 1.0 + MARGIN, 0.0)
    an = np.maximum(sim + MARGIN, 0.0)
    logit_p = -ap * (sim - (1.0 - MARGIN)) * GAMMA
    logit_n = an * (sim - MARGIN) * GAMMA

    def lse(x, m):
        if not m.any():
            return -np.inf
        v = x[m].astype(np.float64)
        mx = v.max()
        return mx + np.log(np.exp(v - mx).sum())

    lp, ln_ = lse(logit_p, pos), lse(logit_n, neg)
    cnt_p = max(int(pos.sum()), 1)
    cnt_n = max(int(neg.sum()), 1)
    wp = float(prob[pos].sum()) / cnt_p if pos.any() else 0.0
    wn = float(prob[neg].sum()) / cnt_n if neg.any() else 0.0
    sp = lambda z: z if z == -np.inf and False else np.logaddexp(0.0, z)
    loss = wp * (0.0 if lp == -np.inf else sp(lp)) + wn * (
        0.0 if ln_ == -np.inf else sp(ln_)
    )
    return np.float32(loss)


def kernel(similarity_matrix, clusters):
    sim = np.asarray(similarity_matrix, dtype=np.float32)
    clu = np.asarray(clusters, dtype=np.float32)

    one_hot = (
        clu.shape == (N, C)
        and sim.shape == (N, N)
        and np.all((clu == 0.0) | (clu == 1.0))
        and np.all(clu.sum(axis=1) == 1.0)
    )
    if not one_hot or float(np.abs(sim).max()) > 1.2:
        return _reference_host(sim, clu)

    cid = clu.argmax(axis=1).astype(np.int64)

    from concourse.bass_utils import run_bass_kernel_spmd

    nc = _get_module()
    in_maps = make_in_maps(sim, cid)
    res = run_bass_kernel_spmd(nc, in_maps, list(range(NCORES)))
    se_arrays = [r["se_out"] for r in res.results]
    loss = _finish(se_arrays, cid)
    if loss is None:
        return _reference_host(sim, clu)
    return loss


# revision 15
# speedup vs baseline: 1.0515x; 1.0515x over previous
"""CircleLoss forward on 8 Trainium2 NeuronCores (Bass/Tile).

Math
----
reference computes, with MARGIN=0.4, GAMMA=80:
    prob = clusters @ clusters.T            (binary when clusters is one-hot)
    pos  = strict-upper & (prob > 0)        (same-cluster pairs, j > i)
    neg  = strict-upper & (prob <= 0)
    logit_p = -relu(1.4 - sim) * (sim - 0.6) * 80
    loss = wp_mean * softplus(lse(logit_p over pos))
         + wn_mean * softplus(lse(logit_n over neg))

With one-hot clusters, prob is exactly {0,1}:
    wn_mean = sum(prob over prob<=0)/cnt = 0       -> neg branch vanishes
    wp_mean = cnt_p/cnt_p = 1 (or 0 if no pos pair)
and |sim| < 1.4 (sim = tanh(...)) makes the relu inactive:
    logit_p = 80*(sim-1)^2 - 12.8
So: loss = softplus( log sum_{pos} exp(80*(sim-1)^2 - 12.8) ).

Since (sim-1)^2 <= 4 for sim in [-1, 1], exp(80*sq - 320) <= 1 never
overflows; we use the fixed offset 320 instead of a data max and the
host adds it back:  lse = ln(S) + (320 - 12.8).

Device kernel (SPMD, identical program on 8 cores)
--------------------------------------------------
Core c owns rows [512c, 512c+512), processed as 4 tiles of 128 rows.
Per [128, 4096] tile:
  DVE : em  = (cid_cols == cid_row)                 bf16, 4x mode
  GPS : em2 = affine_select(em, keep j' > p+128t)   strict-upper mask
  ACT : sq  = Square(sim - 1)                       f32
  ACT : e   = Exp(80*sq - 320)                      f32 -> bf16
  DVE : se  = reduce_add(e * em2)                   -> [128, 1] f32
Host sums the 8*[128,4] partials (f64) and applies softplus.

The affine_select base must be a compile-time constant, but the strict
upper triangle depends on the core's global row offset 512c. Fix: each
core's shard is column-ROTATED by -512c (host-side np.roll), so rotated
column j' maps to original j = (j'+512c) % 4096 and the mask condition
becomes j' > 128t + p -- identical on every core. Rotated-in columns
with original j < 512c are always below the diagonal for this core's
rows; the host overwrites their cluster-id with a sentinel (64) so the
equality mask kills them.
"""

import numpy as np

N = 4096
C = 64
NCORES = 8
RPC = N // NCORES          # rows per core = 512
P = 128                    # partitions per tile
MARGIN = 0.4
GAMMA = 80.0
EXP_OFFSET = 320.0         # exp(GAMMA*sq - EXP_OFFSET); sq <= 4 -> arg <= 0
LSE_BACK = EXP_OFFSET - GAMMA * (1.0 - MARGIN) ** 2 * 0.0 - 12.8
# logit = 80*sq - 12.8 ; e = exp(80*sq - 320) = exp(logit - 307.2)
LSE_BACK = EXP_OFFSET - 12.8

_CACHE = {}


def _build_module(n, ncores, rpc):
    """Build the SPMD Bass module (identical program for every core)."""
    import concourse.bacc as bacc
    import concourse.bass as bass
    import concourse.mybir as mybir
    import concourse.tile as tile
    from contextlib import ExitStack

    p = P
    tiles = rpc // p
    assert rpc % p == 0

    nc = bacc.Bacc(
        "TRN2",
        target_bir_lowering=False,
        debug=False,
        num_devices=ncores,
    )
    f32 = mybir.dt.float32
    bf16 = mybir.dt.bfloat16

    # activation() lowers float biases through the const-AP database; only
    # 0.0/1.0 are pre-registered, so register the two biases we use.
    for val in (-1.0, -EXP_OFFSET):
        t = nc.alloc_sbuf_tensor(f"const-f32-{val}", [P, 1], f32)
        nc.gpsimd.memset(t.ap(), val)
        nc.const_aps.aps[(f32, val)] = t.ap()
    nc.all_engine_barrier()

    sim_in = nc.dram_tensor("simrot", [rpc, n], f32, kind="ExternalInput").ap()
    cid_in = nc.dram_tensor("cidrot", [1, n], bf16, kind="ExternalInput").ap()
    cidrow_in = nc.dram_tensor("cidrow", [p, tiles], f32, kind="ExternalInput").ap()
    out = nc.dram_tensor("se_out", [p, tiles + 1], f32, kind="ExternalOutput").ap()

    MASKV = -1.25e7  # additive mask; *GAMMA in exp scale -> exp(-1e9) = 0

    with tile.TileContext(nc) as tc, ExitStack() as ctx:
        consts = ctx.enter_context(tc.tile_pool(name="consts", bufs=1))
        sim_pool = ctx.enter_context(tc.tile_pool(name="sim", bufs=3))
        em_pool = ctx.enter_context(tc.tile_pool(name="em", bufs=2))
        sq_pool = ctx.enter_context(tc.tile_pool(name="sq", bufs=2))
        arg_pool = ctx.enter_context(tc.tile_pool(name="arg", bufs=2))
        e_pool = ctx.enter_context(tc.tile_pool(name="e", bufs=2))

        # DMA order on the single queue: small constants, then tile 0 in
        # halves (compute on the first half starts at ~half-transfer)
        cid128 = consts.tile([p, n], bf16)
        nc.sync.dma_start(out=cid128[:], in_=cid_in.partition_broadcast(p))
        cidrow = [
            consts.tile([p, 1], f32, name=f"cr{t}", tag=f"cr{t}")
            for t in range(tiles)
        ]
        for t in range(tiles):
            nc.sync.dma_start(out=cidrow[t][:], in_=cidrow_in[:, t : t + 1])
        se = consts.tile([p, tiles + 1], f32)

        h = n // 2
        sim0 = sim_pool.tile([p, n], f32, name="sim0", tag="sim")
        nc.sync.dma_start(out=sim0[:, 0:h], in_=sim_in[0:p, 0:h])
        nc.sync.dma_start(out=sim0[:, h:n], in_=sim_in[0:p, h:n])

        for t in range(tiles):
            if t == 0:
                sim_t = sim0
            else:
                sim_t = sim_pool.tile([p, n], f32, name=f"sim{t}", tag="sim")
                nc.sync.dma_start(
                    out=sim_t[:], in_=sim_in[t * p : (t + 1) * p, :]
                )

            # additive mask: 0 where same cluster, -1.25e7 otherwise
            em = em_pool.tile([p, n], bf16)
            nc.vector.tensor_scalar(
                em[:], cid128[:], cidrow[t][:], MASKV,
                mybir.AluOpType.not_equal, mybir.AluOpType.mult,
            )
            # strict-upper triangle: after rotation only the first
            # 128*(t+1) columns can violate j' > p + 128t -- patch in place
            w = p * (t + 1)
            nc.gpsimd.affine_select(
                out=em[:, 0:w], in_=em[:, 0:w],
                pattern=[[1, w]],
                compare_op=mybir.AluOpType.is_gt,
                fill=MASKV,
                base=-(t * p),
                channel_multiplier=-1,
            )

            sq = sq_pool.tile([p, n], f32)
            argm = arg_pool.tile([p, n], f32)
            e = e_pool.tile([p, n], bf16)
            # tile 0 ramps the pipeline: process in column halves so the
            # first exp (and ACT saturation) starts as early as possible
            if t == 0:
                spans = [(0, h, se[:, 0:1]), (h, n, se[:, tiles : tiles + 1])]
            else:
                spans = [(0, n, se[:, t : t + 1])]
            for lo, hi, acc in spans:
                nc.scalar.activation(
                    sq[:, lo:hi], sim_t[:, lo:hi],
                    mybir.ActivationFunctionType.Square,
                    bias=-1.0, scale=1.0,
                )
                nc.vector.tensor_tensor(
                    argm[:, lo:hi], sq[:, lo:hi], em[:, lo:hi],
                    mybir.AluOpType.add,
                )
                # exp with fused row-accumulate; individual row sums are
                # never needed (fixed offset), so the free-dim accum is the
                # whole per-partition contribution of this span
                nc.scalar.activation(
                    e[:, lo:hi], argm[:, lo:hi],
                    mybir.ActivationFunctionType.Exp,
                    bias=-EXP_OFFSET, scale=GAMMA,
                    accum_out=acc,
                )

        nc.sync.dma_start(out=out, in_=se[:])

    nc.compile()
    return nc


def _get_module(n=N, ncores=NCORES, rpc=RPC):
    key = (n, ncores, rpc)
    if key not in _CACHE:
        _CACHE[key] = _build_module(n, ncores, rpc)
    return _CACHE[key]


def make_in_maps(sim, cid, n=N, ncores=NCORES, rpc=RPC):
    """Per-core rotated shards + cid vectors (see module docstring)."""
    import ml_dtypes

    tiles = rpc // P
    in_maps = []
    for c in range(ncores):
        off = c * rpc
        shard = np.roll(sim[off : off + rpc, :], -off, axis=1)
        cidrot = np.roll(cid, -off)
        if off:
            cidrot[n - off :] = C  # sentinel: wrapped cols are below-diagonal
        cidrow = cid[off : off + rpc].reshape(tiles, P).T  # [P, tiles]
        in_maps.append(
            {
                "simrot": np.ascontiguousarray(shard, dtype=np.float32),
                "cidrot": cidrot.reshape(1, n).astype(ml_dtypes.bfloat16),
                "cidrow": np.ascontiguousarray(cidrow).astype(np.float32),
            }
        )
    return in_maps


def _finish(se_arrays, cid):
    """Merge per-core partial sums into the loss (host, f64)."""
    counts = np.bincount(cid, minlength=C)
    cnt_p = int((counts * (counts - 1) // 2).sum())
    if cnt_p == 0:
        return np.float32(0.0)
    S = float(sum(np.asarray(a, dtype=np.float64).sum() for a in se_arrays))
    if not (S > 1e-35):
        return None  # degenerate: all pos terms underflowed; caller falls back
    lse = np.log(S) + LSE_BACK
    loss = np.logaddexp(0.0, lse)  # softplus
    return np.float32(loss)


def _reference_host(sim, clu):
    """Exact fallback (general inputs), numpy float32 to match reference."""
    sim = sim.astype(np.float32)
    prob = (clu @ clu.T).astype(np.float32)
    upper = np.triu(np.ones(sim.shape, dtype=bool), k=1)
    pos = upper & (prob > 0)
    neg = upper & (prob <= 0)
    ap = np.maximum(-sim + 1.0 + MARGIN, 0.0)
    an = np.maximum(sim + MARGIN, 0.0)
    logit_p = -ap * (sim - (1.0 - MARGIN)) * GAMMA
    logit_n = an * (sim - MARGIN) * GAMMA

    def lse(x, m):
        if not m.any():
            return -np.inf
        v = x[m].astype(np.float64)
        mx = v.max()
        return mx + np.log(np.exp(v - mx).sum())

    lp, ln_ = lse(logit_p, pos), lse(logit_n, neg)
    cnt_p = max(int(pos.sum()), 1)
    cnt_n = max(int(neg.sum()), 1)
    wp = float(prob[pos].sum()) / cnt_p if pos.any() else 0.0
    wn = float(prob[neg].sum()) / cnt_n if neg.any() else 0.0
    sp = lambda z: z if z == -np.inf and False else np.logaddexp(0.0, z)
    loss = wp * (0.0 if lp == -np.inf else sp(lp)) + wn * (
        0.0 if ln_ == -np.inf else sp(ln_)
    )
    return np.float32(loss)


def kernel(similarity_matrix, clusters):
    sim = np.asarray(similarity_matrix, dtype=np.float32)
    clu = np.asarray(clusters, dtype=np.float32)

    one_hot = (
        clu.shape == (N, C)
        and sim.shape == (N, N)
        and np.all((clu == 0.0) | (clu == 1.0))
        and np.all(clu.sum(axis=1) == 1.0)
    )
    if not one_hot or float(np.abs(sim).max()) > 1.2:
        return _reference_host(sim, clu)

    cid = clu.argmax(axis=1).astype(np.int64)

    from concourse.bass_utils import run_bass_kernel_spmd

    nc = _get_module()
    in_maps = make_in_maps(sim, cid)
    res = run_bass_kernel_spmd(nc, in_maps, list(range(NCORES)))
    se_arrays = [r["se_out"] for r in res.results]
    loss = _finish(se_arrays, cid)
    if loss is None:
        return _reference_host(sim, clu)
    return loss


# revision 16
# speedup vs baseline: 1.0565x; 1.0047x over previous
"""CircleLoss forward on 8 Trainium2 NeuronCores (Bass/Tile).

Math
----
reference computes, with MARGIN=0.4, GAMMA=80:
    prob = clusters @ clusters.T            (binary when clusters is one-hot)
    pos  = strict-upper & (prob > 0)        (same-cluster pairs, j > i)
    neg  = strict-upper & (prob <= 0)
    logit_p = -relu(1.4 - sim) * (sim - 0.6) * 80
    loss = wp_mean * softplus(lse(logit_p over pos))
         + wn_mean * softplus(lse(logit_n over neg))

With one-hot clusters, prob is exactly {0,1}:
    wn_mean = sum(prob over prob<=0)/cnt = 0       -> neg branch vanishes
    wp_mean = cnt_p/cnt_p = 1 (or 0 if no pos pair)
and |sim| < 1.4 (sim = tanh(...)) makes the relu inactive:
    logit_p = 80*(sim-1)^2 - 12.8
So: loss = softplus( log sum_{pos} exp(80*(sim-1)^2 - 12.8) ).

Since (sim-1)^2 <= 4 for sim in [-1, 1], exp(80*sq - 320) <= 1 never
overflows; we use the fixed offset 320 instead of a data max and the
host adds it back:  lse = ln(S) + (320 - 12.8).

Device kernel (SPMD, identical program on 8 cores)
--------------------------------------------------
Core c owns rows [512c, 512c+512), processed as 4 tiles of 128 rows.
Per [128, 4096] tile:
  DVE : em  = (cid_cols == cid_row)                 bf16, 4x mode
  GPS : em2 = affine_select(em, keep j' > p+128t)   strict-upper mask
  ACT : sq  = Square(sim - 1)                       f32
  ACT : e   = Exp(80*sq - 320)                      f32 -> bf16
  DVE : se  = reduce_add(e * em2)                   -> [128, 1] f32
Host sums the 8*[128,4] partials (f64) and applies softplus.

The affine_select base must be a compile-time constant, but the strict
upper triangle depends on the core's global row offset 512c. Fix: each
core's shard is column-ROTATED by -512c (host-side np.roll), so rotated
column j' maps to original j = (j'+512c) % 4096 and the mask condition
becomes j' > 128t + p -- identical on every core. Rotated-in columns
with original j < 512c are always below the diagonal for this core's
rows; the host overwrites their cluster-id with a sentinel (64) so the
equality mask kills them.
"""

import numpy as np

N = 4096
C = 64
NCORES = 8
RPC = N // NCORES          # rows per core = 512
P = 128                    # partitions per tile
MARGIN = 0.4
GAMMA = 80.0
EXP_OFFSET = 320.0         # exp(GAMMA*sq - EXP_OFFSET); sq <= 4 -> arg <= 0
LSE_BACK = EXP_OFFSET - GAMMA * (1.0 - MARGIN) ** 2 * 0.0 - 12.8
# logit = 80*sq - 12.8 ; e = exp(80*sq - 320) = exp(logit - 307.2)
LSE_BACK = EXP_OFFSET - 12.8

_CACHE = {}


def _build_module(n, ncores, rpc):
    """Build the SPMD Bass module (identical program for every core)."""
    import concourse.bacc as bacc
    import concourse.bass as bass
    import concourse.mybir as mybir
    import concourse.tile as tile
    from contextlib import ExitStack

    p = P
    tiles = rpc // p
    assert rpc % p == 0

    nc = bacc.Bacc(
        "TRN2",
        target_bir_lowering=False,
        debug=False,
        num_devices=ncores,
    )
    f32 = mybir.dt.float32
    bf16 = mybir.dt.bfloat16

    # activation() lowers float biases through the const-AP database; only
    # 0.0/1.0 are pre-registered, so register the two biases we use.
    for val in (-1.0, -EXP_OFFSET):
        t = nc.alloc_sbuf_tensor(f"const-f32-{val}", [P, 1], f32)
        nc.gpsimd.memset(t.ap(), val)
        nc.const_aps.aps[(f32, val)] = t.ap()
    nc.all_engine_barrier()

    sim_in = nc.dram_tensor("simrot", [rpc, n], f32, kind="ExternalInput").ap()
    cid_in = nc.dram_tensor("cidrot", [1, n], bf16, kind="ExternalInput").ap()
    cidrow_in = nc.dram_tensor("cidrow", [p, tiles], f32, kind="ExternalInput").ap()
    out = nc.dram_tensor("se_out", [p, tiles + 1], f32, kind="ExternalOutput").ap()

    MASKV = -1.25e7  # additive mask; *GAMMA in exp scale -> exp(-1e9) = 0

    with tile.TileContext(nc) as tc, ExitStack() as ctx:
        consts = ctx.enter_context(tc.tile_pool(name="consts", bufs=1))
        sim_pool = ctx.enter_context(tc.tile_pool(name="sim", bufs=3))
        em_pool = ctx.enter_context(tc.tile_pool(name="em", bufs=2))
        sq_pool = ctx.enter_context(tc.tile_pool(name="sq", bufs=2))
        arg_pool = ctx.enter_context(tc.tile_pool(name="arg", bufs=2))
        e_pool = ctx.enter_context(tc.tile_pool(name="e", bufs=2))

        # Two HWDGE rings (sync=qSP, scalar=qAct): stream sim on sync; put
        # the cid broadcast + tile0's second half on the scalar ring so the
        # ramp-critical transfers run in parallel.
        cid128 = consts.tile([p, n], bf16)
        nc.scalar.dma_start(out=cid128[:], in_=cid_in.partition_broadcast(p))
        cidrow = [
            consts.tile([p, 1], f32, name=f"cr{t}", tag=f"cr{t}")
            for t in range(tiles)
        ]
        for t in range(tiles):
            nc.sync.dma_start(out=cidrow[t][:], in_=cidrow_in[:, t : t + 1])
        se = consts.tile([p, tiles + 1], f32)

        h = n // 2
        sim0 = sim_pool.tile([p, n], f32, name="sim0", tag="sim")
        nc.sync.dma_start(out=sim0[:, 0:h], in_=sim_in[0:p, 0:h])
        nc.scalar.dma_start(out=sim0[:, h:n], in_=sim_in[0:p, h:n])

        for t in range(tiles):
            if t == 0:
                sim_t = sim0
            else:
                sim_t = sim_pool.tile([p, n], f32, name=f"sim{t}", tag="sim")
                nc.sync.dma_start(
                    out=sim_t[:], in_=sim_in[t * p : (t + 1) * p, :]
                )

            # additive mask: 0 where same cluster, -1.25e7 otherwise
            em = em_pool.tile([p, n], bf16)
            nc.vector.tensor_scalar(
                em[:], cid128[:], cidrow[t][:], MASKV,
                mybir.AluOpType.not_equal, mybir.AluOpType.mult,
            )
            # strict-upper triangle: after rotation only the first
            # 128*(t+1) columns can violate j' > p + 128t -- patch in place
            w = p * (t + 1)
            nc.gpsimd.affine_select(
                out=em[:, 0:w], in_=em[:, 0:w],
                pattern=[[1, w]],
                compare_op=mybir.AluOpType.is_gt,
                fill=MASKV,
                base=-(t * p),
                channel_multiplier=-1,
            )

            sq = sq_pool.tile([p, n], f32)
            argm = arg_pool.tile([p, n], f32)
            e = e_pool.tile([p, n], bf16)
            # tile 0 ramps the pipeline: process in column halves so the
            # first exp (and ACT saturation) starts as early as possible
            if t == 0:
                spans = [(0, h, se[:, 0:1]), (h, n, se[:, tiles : tiles + 1])]
            else:
                spans = [(0, n, se[:, t : t + 1])]
            for lo, hi, acc in spans:
                nc.scalar.activation(
                    sq[:, lo:hi], sim_t[:, lo:hi],
                    mybir.ActivationFunctionType.Square,
                    bias=-1.0, scale=1.0,
                )
                nc.vector.tensor_tensor(
                    argm[:, lo:hi], sq[:, lo:hi], em[:, lo:hi],
                    mybir.AluOpType.add,
                )
                # exp with fused row-accumulate; individual row sums are
                # never needed (fixed offset), so the free-dim accum is the
                # whole per-partition contribution of this span
                nc.scalar.activation(
                    e[:, lo:hi], argm[:, lo:hi],
                    mybir.ActivationFunctionType.Exp,
                    bias=-EXP_OFFSET, scale=GAMMA,
                    accum_out=acc,
                )

        nc.sync.dma_start(out=out, in_=se[:])

    nc.compile()
    return nc


def _get_module(n=N, ncores=NCORES, rpc=RPC):
    key = (n, ncores, rpc)
    if key not in _CACHE:
        _CACHE[key] = _build_module(n, ncores, rpc)
    return _CACHE[key]


def make_in_maps(sim, cid, n=N, ncores=NCORES, rpc=RPC):
    """Per-core rotated shards + cid vectors (see module docstring)."""
    import ml_dtypes

    tiles = rpc // P
    in_maps = []
    for c in range(ncores):
        off = c * rpc
        shard = np.roll(sim[off : off + rpc, :], -off, axis=1)
        cidrot = np.roll(cid, -off)
        if off:
            cidrot[n - off :] = C  # sentinel: wrapped cols are below-diagonal
        cidrow = cid[off : off + rpc].reshape(tiles, P).T  # [P, tiles]
        in_maps.append(
            {
                "simrot": np.ascontiguousarray(shard, dtype=np.float32),
                "cidrot": cidrot.reshape(1, n).astype(ml_dtypes.bfloat16),
                "cidrow": np.ascontiguousarray(cidrow).astype(np.float32),
            }
        )
    return in_maps


def _finish(se_arrays, cid):
    """Merge per-core partial sums into the loss (host, f64)."""
    counts = np.bincount(cid, minlength=C)
    cnt_p = int((counts * (counts - 1) // 2).sum())
    if cnt_p == 0:
        return np.float32(0.0)
    S = float(sum(np.asarray(a, dtype=np.float64).sum() for a in se_arrays))
    if not (S > 1e-35):
        return None  # degenerate: all pos terms underflowed; caller falls back
    lse = np.log(S) + LSE_BACK
    loss = np.logaddexp(0.0, lse)  # softplus
    return np.float32(loss)


def _reference_host(sim, clu):
    """Exact fallback (general inputs), numpy float32 to match reference."""
    sim = sim.astype(np.float32)
    prob = (clu @ clu.T).astype(np.float32)
    upper = np.triu(np.ones(sim.shape, dtype=bool), k=1)
    pos = upper & (prob > 0)
    neg = upper & (prob <= 0)
    ap = np.maximum(-sim + 1.0 + MARGIN, 0.0)
    an = np.maximum(sim + MARGIN, 0.0)
    logit_p = -ap * (sim - (1.0 - MARGIN)) * GAMMA
    logit_n = an * (sim - MARGIN) * GAMMA

    def lse(x, m):
        if not m.any():
            return -np.inf
        v = x[m].astype(np.float64)
        mx = v.max()
        return mx + np.log(np.exp(v - mx).sum())

    lp, ln_ = lse(logit_p, pos), lse(logit_n, neg)
    cnt_p = max(int(pos.sum()), 1)
    cnt_n = max(int(neg.sum()), 1)
    wp = float(prob[pos].sum()) / cnt_p if pos.any() else 0.0
    wn = float(prob[neg].sum()) / cnt_n if neg.any() else 0.0
    sp = lambda z: z if z == -np.inf and False else np.logaddexp(0.0, z)
    loss = wp * (0.0 if lp == -np.inf else sp(lp)) + wn * (
        0.0 if ln_ == -np.inf else sp(ln_)
    )
    return np.float32(loss)


def kernel(similarity_matrix, clusters):
    sim = np.asarray(similarity_matrix, dtype=np.float32)
    clu = np.asarray(clusters, dtype=np.float32)

    one_hot = (
        clu.shape == (N, C)
        and sim.shape == (N, N)
        and np.all((clu == 0.0) | (clu == 1.0))
        and np.all(clu.sum(axis=1) == 1.0)
    )
    if not one_hot or float(np.abs(sim).max()) > 1.2:
        return _reference_host(sim, clu)

    cid = clu.argmax(axis=1).astype(np.int64)

    from concourse.bass_utils import run_bass_kernel_spmd

    nc = _get_module()
    in_maps = make_in_maps(sim, cid)
    res = run_bass_kernel_spmd(nc, in_maps, list(range(NCORES)))
    se_arrays = [r["se_out"] for r in res.results]
    loss = _finish(se_arrays, cid)
    if loss is None:
        return _reference_host(sim, clu)
    return loss


# revision 22
# speedup vs baseline: 1.0876x; 1.0294x over previous
"""CircleLoss forward on 8 Trainium2 NeuronCores (Bass/Tile).

Math
----
reference computes, with MARGIN=0.4, GAMMA=80:
    prob = clusters @ clusters.T            (binary when clusters is one-hot)
    pos  = strict-upper & (prob > 0)        (same-cluster pairs, j > i)
    neg  = strict-upper & (prob <= 0)
    logit_p = -relu(1.4 - sim) * (sim - 0.6) * 80
    loss = wp_mean * softplus(lse(logit_p over pos))
         + wn_mean * softplus(lse(logit_n over neg))

With one-hot clusters, prob is exactly {0,1}:
    wn_mean = sum(prob over prob<=0)/cnt = 0       -> neg branch vanishes
    wp_mean = cnt_p/cnt_p = 1 (or 0 if no pos pair)
and |sim| < 1.4 (sim = tanh(...)) makes the relu inactive:
    logit_p = 80*(sim-1)^2 - 12.8
So: loss = softplus( log sum_{pos} exp(80*(sim-1)^2 - 12.8) ).

Since (sim-1)^2 <= 4 for sim in [-1, 1], exp(80*sq - 320) <= 1 never
overflows; we use the fixed offset 320 instead of a data max and the
host adds it back:  lse = ln(S) + (320 - 12.8).

Device kernel (SPMD, identical program on 8 cores)
--------------------------------------------------
Core c owns rows [512c, 512c+512), processed as 4 tiles of 128 rows.
Per [128, 4096] tile:
  DVE : em  = (cid_cols == cid_row)                 bf16, 4x mode
  GPS : em2 = affine_select(em, keep j' > p+128t)   strict-upper mask
  ACT : sq  = Square(sim - 1)                       f32
  ACT : e   = Exp(80*sq - 320)                      f32 -> bf16
  DVE : se  = reduce_add(e * em2)                   -> [128, 1] f32
Host sums the 8*[128,4] partials (f64) and applies softplus.

The affine_select base must be a compile-time constant, but the strict
upper triangle depends on the core's global row offset 512c. Fix: each
core's shard is column-ROTATED by -512c (host-side np.roll), so rotated
column j' maps to original j = (j'+512c) % 4096 and the mask condition
becomes j' > 128t + p -- identical on every core. Rotated-in columns
with original j < 512c are always below the diagonal for this core's
rows; the host overwrites their cluster-id with a sentinel (64) so the
equality mask kills them.
"""

import numpy as np

N = 4096
C = 64
NCORES = 8
RPC = N // NCORES          # rows per core = 512
P = 128                    # partitions per tile
MARGIN = 0.4
GAMMA = 80.0
EXP_OFFSET = 320.0         # exp(GAMMA*sq - EXP_OFFSET); sq <= 4 -> arg <= 0
LSE_BACK = EXP_OFFSET - GAMMA * (1.0 - MARGIN) ** 2 * 0.0 - 12.8
# logit = 80*sq - 12.8 ; e = exp(80*sq - 320) = exp(logit - 307.2)
LSE_BACK = EXP_OFFSET - 12.8

_CACHE = {}


def _build_module(n, ncores, rpc):
    """Build the SPMD Bass module (identical program for every core)."""
    import concourse.bacc as bacc
    import concourse.bass as bass
    import concourse.mybir as mybir
    import concourse.tile as tile
    from contextlib import ExitStack

    p = P
    tiles = rpc // p
    assert rpc % p == 0

    nc = bacc.Bacc(
        "TRN2",
        target_bir_lowering=False,
        debug=False,
        num_devices=ncores,
    )
    f32 = mybir.dt.float32
    bf16 = mybir.dt.bfloat16

    # activation() lowers float biases through the const-AP database; only
    # 0.0/1.0 are pre-registered, so register the two biases we use.
    for val in (-1.0, -EXP_OFFSET):
        t = nc.alloc_sbuf_tensor(f"const-f32-{val}", [P, 1], f32)
        nc.gpsimd.memset(t.ap(), val)
        nc.const_aps.aps[(f32, val)] = t.ap()
    nc.all_engine_barrier()

    sim_in = nc.dram_tensor("simrot", [rpc, n], f32, kind="ExternalInput").ap()
    cid_in = nc.dram_tensor("cidrot", [1, n], bf16, kind="ExternalInput").ap()
    cidrow_in = nc.dram_tensor("cidrow", [p, tiles], f32, kind="ExternalInput").ap()
    h = n // 2
    # span plan: (lo, hi, square-engine). Tiles 0/1 ramp with their
    # second-half squares on DVE (fills DVE bubbles while ACT warms up);
    # tile 3 is split so the final exp/accum lands earlier.
    span_plan = {
        0: [(0, h, "act"), (h, n, "dve")],
        1: [(0, h, "act"), (h, n, "dve")],
        2: [(0, n, "act")],
        3: [(0, h, "act"), (h, n, "act")],
    }
    if tiles != 4:  # reduced-size sim builds
        span_plan = {t: [(0, n, "act")] for t in range(tiles)}
    n_spans = sum(len(v) for v in span_plan.values())

    out = nc.dram_tensor("se_out", [p, n_spans], f32, kind="ExternalOutput").ap()

    MASKV = -1.25e7  # additive mask; *GAMMA in exp scale -> exp(-1e9) = 0

    with tile.TileContext(nc) as tc, ExitStack() as ctx:
        consts = ctx.enter_context(tc.tile_pool(name="consts", bufs=1))
        sim_pool = ctx.enter_context(tc.tile_pool(name="sim", bufs=3))
        em_pool = ctx.enter_context(tc.tile_pool(name="em", bufs=2))
        sq_pool = ctx.enter_context(tc.tile_pool(name="sq", bufs=2))
        arg_pool = ctx.enter_context(tc.tile_pool(name="arg", bufs=2))
        e_pool = ctx.enter_context(tc.tile_pool(name="e", bufs=2))
        d_pool = ctx.enter_context(tc.tile_pool(name="d", bufs=2))

        # Two HWDGE rings (sync=qSP, scalar=qAct): stream sim on sync; put
        # the cid broadcast + tile0's second half on the scalar ring so the
        # ramp-critical transfers run in parallel.
        cid128 = consts.tile([p, n], bf16)
        nc.scalar.dma_start(out=cid128[:], in_=cid_in.partition_broadcast(p))
        cidrow = [
            consts.tile([p, 1], f32, name=f"cr{t}", tag=f"cr{t}")
            for t in range(tiles)
        ]
        for t in range(tiles):
            nc.sync.dma_start(out=cidrow[t][:], in_=cidrow_in[:, t : t + 1])
        se = consts.tile([p, n_spans], f32)

        sim0 = sim_pool.tile([p, n], f32, name="sim0", tag="sim")
        nc.sync.dma_start(out=sim0[:, 0:h], in_=sim_in[0:p, 0:h])
        nc.scalar.dma_start(out=sim0[:, h:n], in_=sim_in[0:p, h:n])

        acc_col = 0
        for t in range(tiles):
            if t == 0:
                sim_t = sim0
            else:
                sim_t = sim_pool.tile([p, n], f32, name=f"sim{t}", tag="sim")
                nc.sync.dma_start(
                    out=sim_t[:], in_=sim_in[t * p : (t + 1) * p, :]
                )

            # additive mask: 0 where same cluster, -1.25e7 otherwise
            em = em_pool.tile([p, n], bf16)
            nc.vector.tensor_scalar(
                em[:], cid128[:], cidrow[t][:], MASKV,
                mybir.AluOpType.not_equal, mybir.AluOpType.mult,
            )
            # strict-upper triangle: after rotation only the first
            # 128*(t+1) columns can violate j' > p + 128t -- patch in place
            w = p * (t + 1)
            nc.gpsimd.affine_select(
                out=em[:, 0:w], in_=em[:, 0:w],
                pattern=[[1, w]],
                compare_op=mybir.AluOpType.is_gt,
                fill=MASKV,
                base=-(t * p),
                channel_multiplier=-1,
            )

            sq = sq_pool.tile([p, n], f32)
            argm = arg_pool.tile([p, n], f32)
            e = e_pool.tile([p, n], bf16)
            for lo, hi, sq_eng in span_plan[t]:
                if sq_eng == "dve":
                    dd = d_pool.tile([p, hi - lo], f32, name=f"d{t}", tag="d")
                    nc.vector.tensor_scalar(
                        dd[:], sim_t[:, lo:hi], 1.0, None,
                        mybir.AluOpType.subtract,
                    )
                    nc.vector.tensor_tensor(
                        sq[:, lo:hi], dd[:], dd[:], mybir.AluOpType.mult
                    )
                else:
                    nc.scalar.activation(
                        sq[:, lo:hi], sim_t[:, lo:hi],
                        mybir.ActivationFunctionType.Square,
                        bias=-1.0, scale=1.0,
                    )
                nc.vector.tensor_tensor(
                    argm[:, lo:hi], sq[:, lo:hi], em[:, lo:hi],
                    mybir.AluOpType.add,
                )
                # exp with fused row-accumulate; individual row sums are
                # never needed (fixed offset), so the free-dim accum is the
                # whole per-partition contribution of this span
                nc.scalar.activation(
                    e[:, lo:hi], argm[:, lo:hi],
                    mybir.ActivationFunctionType.Exp,
                    bias=-EXP_OFFSET, scale=GAMMA,
                    accum_out=se[:, acc_col : acc_col + 1],
                )
                acc_col += 1

        nc.sync.dma_start(out=out, in_=se[:])

    nc.compile()
    return nc


def _get_module(n=N, ncores=NCORES, rpc=RPC):
    key = (n, ncores, rpc)
    if key not in _CACHE:
        _CACHE[key] = _build_module(n, ncores, rpc)
    return _CACHE[key]


def make_in_maps(sim, cid, n=N, ncores=NCORES, rpc=RPC):
    """Per-core rotated shards + cid vectors (see module docstring)."""
    import ml_dtypes

    tiles = rpc // P
    in_maps = []
    for c in range(ncores):
        off = c * rpc
        shard = np.roll(sim[off : off + rpc, :], -off, axis=1)
        cidrot = np.roll(cid, -off)
        if off:
            cidrot[n - off :] = C  # sentinel: wrapped cols are below-diagonal
        cidrow = cid[off : off + rpc].reshape(tiles, P).T  # [P, tiles]
        in_maps.append(
            {
                "simrot": np.ascontiguousarray(shard, dtype=np.float32),
                "cidrot": cidrot.reshape(1, n).astype(ml_dtypes.bfloat16),
                "cidrow": np.ascontiguousarray(cidrow).astype(np.float32),
            }
        )
    return in_maps


def _finish(se_arrays, cid):
    """Merge per-core partial sums into the loss (host, f64)."""
    counts = np.bincount(cid, minlength=C)
    cnt_p = int((counts * (counts - 1) // 2).sum())
    if cnt_p == 0:
        return np.float32(0.0)
    S = float(sum(np.asarray(a, dtype=np.float64).sum() for a in se_arrays))
    if not (S > 1e-35):
        return None  # degenerate: all pos terms underflowed; caller falls back
    lse = np.log(S) + LSE_BACK
    loss = np.logaddexp(0.0, lse)  # softplus
    return np.float32(loss)


def _reference_host(sim, clu):
    """Exact fallback (general inputs), numpy float32 to match reference."""
    sim = sim.astype(np.float32)
    prob = (clu @ clu.T).astype(np.float32)
    upper = np.triu(np.ones(sim.shape, dtype=bool), k=1)
    pos = upper & (prob > 0)
    neg = upper & (prob <= 0)
    ap = np.maximum(-sim + 1.0 + MARGIN, 0.0)
    an = np.maximum(sim + MARGIN, 0.0)
    logit_p = -ap * (sim - (1.0 - MARGIN)) * GAMMA
    logit_n = an * (sim - MARGIN) * GAMMA

    def lse(x, m):
        if not m.any():
            return -np.inf
        v = x[m].astype(np.float64)
        mx = v.max()
        return mx + np.log(np.exp(v - mx).sum())

    lp, ln_ = lse(logit_p, pos), lse(logit_n, neg)
    cnt_p = max(int(pos.sum()), 1)
    cnt_n = max(int(neg.sum()), 1)
    wp = float(prob[pos].sum()) / cnt_p if pos.any() else 0.0
    wn = float(prob[neg].sum()) / cnt_n if neg.any() else 0.0
    sp = lambda z: z if z == -np.inf and False else np.logaddexp(0.0, z)
    loss = wp * (0.0 if lp == -np.inf else sp(lp)) + wn * (
        0.0 if ln_ == -np.inf else sp(ln_)
    )
    return np.float32(loss)


def kernel(similarity_matrix, clusters):
    sim = np.asarray(similarity_matrix, dtype=np.float32)
    clu = np.asarray(clusters, dtype=np.float32)

    one_hot = (
        clu.shape == (N, C)
        and sim.shape == (N, N)
        and np.all((clu == 0.0) | (clu == 1.0))
        and np.all(clu.sum(axis=1) == 1.0)
    )
    if not one_hot or float(np.abs(sim).max()) > 1.2:
        return _reference_host(sim, clu)

    cid = clu.argmax(axis=1).astype(np.int64)

    from concourse.bass_utils import run_bass_kernel_spmd

    nc = _get_module()
    in_maps = make_in_maps(sim, cid)
    res = run_bass_kernel_spmd(nc, in_maps, list(range(NCORES)))
    se_arrays = [r["se_out"] for r in res.results]
    loss = _finish(se_arrays, cid)
    if loss is None:
        return _reference_host(sim, clu)
    return loss
